# revision 1
# baseline (speedup 1.0000x reference)
"""DiffAttn TRN2 kernel: 8-core SPMD (batch x query-half sharding).

Per core (batch b = core//2, query half h = core%2):
  q12 = x[b,qrows] @ w_q12 ; k12 = x[b] @ w_k12 ; v = x[b] @ w_v
  sT_a[s,q] = sum_d kT_a[d,s] * qT_a[d,q]          (scores, keys on partitions)
  e_a = exp(scale * sT_a);  den_a[q] = sum_s e_a[s,q]   (ones-matmul on PE)
  diffT = e_1/den_1 - lam * e_2/den_2                   (DVE, in place)
  out[q,d] = sum_s diffT[s,q] v[s,d];  RMSNorm(out) * (1-lambda_init)

All heavy matmuls run in float32r (full PE rate at moving-dim>=256,
~1.5e-4 relative error vs fp32, measured on HW). k12/v stage through DRAM
(k in (st,dt)-block layout for contiguous attention reads); q12T stays
resident in SBUF. Pool lifetimes are staggered manually so each phase's
weights prefetch during the previous phase (their DMAs only WAR on
already-finished readers), keeping the PE fed across phase boundaries.
"""

import sys

for _p in ("/opt/trn_rl_repo", "/root/.axon_site/_ro/trn_rl_repo"):
    if _p not in sys.path:
        sys.path.append(_p)

import numpy as np

import concourse.bass as bass
import concourse.mybir as mybir
from concourse import bacc
from concourse.bass_utils import run_bass_kernel_spmd
from concourse.tile import TileContext

F32 = mybir.dt.float32
F32R = mybir.dt.float32r
AF = mybir.ActivationFunctionType

D = 1024          # embed dim
S = 2048          # sequence length
B = 4             # batch
NCORES = 8
QH = 1024         # query rows per core (half a sequence)
QB = 512          # query block (matmul moving dim)
NQB = QH // QB    # 2
NDT = D // 128    # 8 contraction tiles
NST = S // 128    # 16 key tiles
LAMBDA_INIT = 0.8
EPS = 1e-5
SCALE = float(D) ** -0.25

_CACHE = {}


def _build_nc():
    nc = bacc.Bacc("TRN2", target_bir_lowering=False, debug=False,
                   num_devices=NCORES)

    xT = nc.declare_dram_parameter("xT", [D, S], F32, isOutput=False)
    xTq = nc.declare_dram_parameter("xTq", [D, QH], F32, isOutput=False)
    wq = nc.declare_dram_parameter("wq", [D, 2 * D], F32, isOutput=False)
    wk = nc.declare_dram_parameter("wk", [D, 2 * D], F32, isOutput=False)
    wv = nc.declare_dram_parameter("wv", [D, D], F32, isOutput=False)
    lams = nc.declare_dram_parameter("lams", [1, 4 * D], F32, isOutput=False)
    out = nc.declare_dram_parameter("out", [QH, D], F32, isOutput=True)

    k_d = nc.dram_tensor("k_d", [2, NST, NDT, 128, 128], F32)
    v_d = nc.dram_tensor("v_d", [S, D], F32)

    # DRAM views
    xT_v = xT.ap().rearrange("(dt p) s -> p dt s", p=128).bitcast(F32R)
    xq_v = xTq.ap().rearrange("(dt p) q -> p dt q", p=128).bitcast(F32R)
    wk_v = wk.ap().rearrange("(dt p) e -> p dt e", p=128).bitcast(F32R)
    wq_v = wq.ap().rearrange("(dt p) e -> p dt e", p=128).bitcast(F32R)
    wv_v = wv.ap().rearrange("(dt p) e -> p dt e", p=128).bitcast(F32R)
    v_st = v_d.ap().rearrange("(st p) e -> st p e", p=128)          # [16,128,D]
    out_v = out.ap().rearrange("(t p) e -> t p e", p=128)           # [8,128,D]

    with TileContext(nc) as tc:
        singles_cm = tc.tile_pool(name="singles", bufs=1)
        singles = singles_cm.__enter__()

        # ---- lambda scalar -----------------------------------------------
        # host packs lams as [q1, q2, k1, k2] so the q/k pairs align at the
        # same base partition for the DVE multiply
        lam_q = singles.tile([64, 32], F32)
        lam_k = singles.tile([64, 32], F32)
        nc.sync.dma_start(
            out=lam_q,
            in_=lams.ap()[:, 0:2 * D].rearrange("o (p f) -> (o p) f", p=64))
        nc.sync.dma_start(
            out=lam_k,
            in_=lams.ap()[:, 2 * D:4 * D].rearrange("o (p f) -> (o p) f", p=64))
        prod = singles.tile([64, 32], F32)
        nc.vector.tensor_mul(prod, lam_q, lam_k)
        rowsum = singles.tile([64, 1], F32)
        nc.vector.tensor_reduce(rowsum, prod, axis=mybir.AxisListType.X,
                                op=mybir.AluOpType.add)
        rs2 = singles.tile([32, 1], F32)
        nc.vector.tensor_copy(rs2, rowsum[32:64, :])
        s12 = singles.tile([1, 2], F32)
        nc.gpsimd.tensor_reduce(s12[:, 0:1], rowsum[0:32, :],
                                axis=mybir.AxisListType.C,
                                op=mybir.AluOpType.add)
        nc.gpsimd.tensor_reduce(s12[:, 1:2], rs2, axis=mybir.AxisListType.C,
                                op=mybir.AluOpType.add)
        e12 = singles.tile([1, 2], F32)
        nc.scalar.activation(e12, s12, AF.Exp)
        lamv = singles.tile([1, 1], F32)
        nc.vector.tensor_sub(lamv, e12[:, 0:1], e12[:, 1:2])
        nc.vector.tensor_scalar_add(lamv, lamv, LAMBDA_INIT)

        ones_f = singles.tile([128, 1], F32)
        nc.vector.memset(ones_f, 1.0)
        ones_r = singles.tile([128, 1], F32R)
        nc.scalar.copy(ones_r, ones_f)
        eps_sb = singles.tile([128, 1], F32)
        nc.vector.memset(eps_sb, EPS)

        # ---- pools with staggered lifetimes ------------------------------
        # left stack: xT -> (wk-stream, kdrain) -> vdrain -> q12 -> attention
        # right stack: wq (whole proj span), wv (until v-proj done), xq
        px_cm = tc.tile_pool(name="px", bufs=1)
        px = px_cm.__enter__()
        pwq_cm = tc.tile_pool(name="pwq", bufs=1, side="right")
        pwq = pwq_cm.__enter__()
        pwv_cm = tc.tile_pool(name="pwv", bufs=1, side="right")
        pwv = pwv_cm.__enter__()
        pwk_cm = tc.tile_pool(name="pwk", bufs=4)   # wk streamed per-et
        pwk = pwk_cm.__enter__()
        kdrain_cm = tc.tile_pool(name="kdrain", bufs=3)
        kdrain = kdrain_cm.__enter__()
        psk_cm = tc.tile_pool(name="psk", bufs=2, space="PSUM")
        psk = psk_cm.__enter__()

        xT_sb = px.tile([128, NDT, S], F32R)
        wq_sb = pwq.tile([128, NDT, 2 * D], F32R)
        wv_sb = pwv.tile([128, NDT, D], F32R)
        # DMA issue order feeds the k-projection's dt-ascending pipeline:
        # xT[0], the first wk tiles, then the remaining xT tiles
        nc.sync.dma_start(out=xT_sb[:, 0, :], in_=xT_v[:, 0, :])
        wkts = {}
        for et in range(3):
            wkts[et] = pwk.tile([128, NDT, 128], F32R, tag="wk",
                                name="wkt", bufs=4)
            nc.sync.dma_start(out=wkts[et],
                              in_=wk_v[:, :, et * 128:(et + 1) * 128])
        for dt in range(1, NDT):
            nc.sync.dma_start(out=xT_sb[:, dt, :], in_=xT_v[:, dt, :])

        # ---- k-projection -> k_d in (attn, st, dt) block layout ----------
        for et in range(2 * NDT):
            if et in wkts:
                wkt = wkts[et]
            else:
                wkt = pwk.tile([128, NDT, 128], F32R, tag="wk", name="wkt",
                               bufs=4)
                nc.sync.dma_start(out=wkt,
                                  in_=wk_v[:, :, et * 128:(et + 1) * 128])
            pk = psk.tile([128, S], F32, name="pk")
            for dt in range(NDT):
                lhsT = wkt[:, dt, :]
                for sc in range(S // 512):
                    nc.tensor.matmul(
                        pk[:, sc * 512:(sc + 1) * 512],
                        lhsT=lhsT,
                        rhs=xT_sb[:, dt, sc * 512:(sc + 1) * 512],
                        start=(dt == 0), stop=(dt == NDT - 1))
            sk = kdrain.tile([128, NST, 128], F32, name="sk")
            nc.scalar.copy(sk, pk.rearrange("p (st s) -> p st s", st=NST))
            a, dtl = divmod(et, NDT)
            nc.sync.dma_start(
                out=k_d.ap()[a, :, dtl, :, :].rearrange("st p s -> p st s"),
                in_=sk)
            if et == 4:
                # prefetch next phases' weights once the critical xT/wk head
                # of the DMA queue has drained
                for dt in range(NDT):
                    nc.sync.dma_start(out=wv_sb[:, dt, :], in_=wv_v[:, dt, :])
            if et == 9:
                for dt in range(NDT // 2):
                    nc.sync.dma_start(out=wq_sb[:, dt, :], in_=wq_v[:, dt, :])
            if et == 12:
                for dt in range(NDT // 2, NDT):
                    nc.sync.dma_start(out=wq_sb[:, dt, :], in_=wq_v[:, dt, :])

        psk_cm.__exit__(None, None, None)
        kdrain_cm.__exit__(None, None, None)
        pwk_cm.__exit__(None, None, None)

        vdrain_cm = tc.tile_pool(name="vdrain", bufs=3)
        vdrain = vdrain_cm.__enter__()
        psv_cm = tc.tile_pool(name="psv", bufs=2, space="PSUM")
        psv = psv_cm.__enter__()

        # ---- v-projection -> v_d [s, e] ----------------------------------
        for st in range(NST):
            pv = psv.tile([128, D], F32, name="pv")
            for dt in range(NDT):
                lhsT = xT_sb[:, dt, st * 128:(st + 1) * 128]
                for oc in range(D // 512):
                    nc.tensor.matmul(
                        pv[:, oc * 512:(oc + 1) * 512],
                        lhsT=lhsT,
                        rhs=wv_sb[:, dt, oc * 512:(oc + 1) * 512],
                        start=(dt == 0), stop=(dt == NDT - 1))
            sv = vdrain.tile([128, D], F32, name="sv")
            nc.scalar.copy(sv, pv)
            nc.sync.dma_start(out=v_st[st], in_=sv)
        # free xT/wv; q12T becomes resident in their space
        psv_cm.__exit__(None, None, None)
        vdrain_cm.__exit__(None, None, None)
        pwv_cm.__exit__(None, None, None)
        px_cm.__exit__(None, None, None)

        pxq_cm = tc.tile_pool(name="pxq", bufs=1, side="right")
        pxq = pxq_cm.__enter__()
        xq_sb = pxq.tile([128, NDT, QH], F32R)
        for dt in range(NDT):
            nc.sync.dma_start(out=xq_sb[:, dt, :], in_=xq_v[:, dt, :])

        # attention k/v stream pools open before q-proj so their first
        # DMAs prefetch during the q-projection window
        kstream_cm = tc.tile_pool(name="kstream", bufs=8)
        kstream = kstream_cm.__enter__()
        vstream_cm = tc.tile_pool(name="vstream", bufs=6)
        vstream = vstream_cm.__enter__()

        q12_cm = tc.tile_pool(name="q12pool", bufs=1)
        q12pool = q12_cm.__enter__()
        q12_sb = q12pool.tile([128, 2 * NDT, QH], F32R)
        psq_cm = tc.tile_pool(name="psq", bufs=2, space="PSUM")
        psq = psq_cm.__enter__()

        # ---- q-projection: writes q12_sb directly (no DRAM staging) ------
        for et in range(2 * NDT):
            pq = psq.tile([128, QH], F32, name="pq")
            for dt in range(NDT):
                lhsT = wq_sb[:, dt, et * 128:(et + 1) * 128]
                for qc in range(QH // 512):
                    nc.tensor.matmul(
                        pq[:, qc * 512:(qc + 1) * 512],
                        lhsT=lhsT,
                        rhs=xq_sb[:, dt, qc * 512:(qc + 1) * 512],
                        start=(dt == 0), stop=(dt == NDT - 1))
            nc.scalar.copy(q12_sb[:, et, :], pq)

        psq_cm.__exit__(None, None, None)
        pxq_cm.__exit__(None, None, None)
        pwq_cm.__exit__(None, None, None)

        # ---- attention ---------------------------------------------------
        with tc.tile_pool(name="eblk", bufs=1) as eblk, \
             tc.tile_pool(name="work", bufs=2) as work, \
             tc.tile_pool(name="pssc", bufs=2, space="PSUM") as pssc, \
             tc.tile_pool(name="psden", bufs=2, space="PSUM") as psden, \
             tc.tile_pool(name="psout", bufs=4, space="PSUM") as psout:
            for bi in range(NQB):
                qs = bi * QB
                eT = {}
                for a in (0, 1):
                    eT[a] = eblk.tile([128, NST, QB], F32R,
                                      tag=f"e{a}", name=f"eT{a}")
                    for st in range(NST):
                        kt = kstream.tile([128, NDT, 128], F32R,
                                          tag="k", name="kt")
                        nc.sync.dma_start(
                            out=kt,
                            in_=k_d.ap()[a, st].rearrange(
                                "dt p s -> p dt s").bitcast(F32R))
                        psc = pssc.tile([128, QB], F32, tag="sc", name="psc")
                        for dt in range(NDT):
                            nc.tensor.matmul(
                                psc,
                                lhsT=kt[:, dt, :],
                                rhs=q12_sb[:, a * NDT + dt, qs:qs + QB],
                                start=(dt == 0), stop=(dt == NDT - 1))
                        nc.scalar.activation(eT[a][:, st, :], psc, AF.Exp,
                                             scale=SCALE)
                    # denominator over s (partition axis) via ones-matmul
                    pden = psden.tile([1, QB], F32, tag="den", name="pden")
                    for st in range(NST):
                        nc.tensor.matmul(pden, lhsT=ones_r,
                                         rhs=eT[a][:, st, :],
                                         start=(st == 0), stop=(st == NST - 1))
                    rden = work.tile([1, QB], F32, tag="rden", name="rden",
                                     bufs=1)
                    nc.vector.reciprocal_approx_fast(rden, pden)
                    if a == 1:
                        nc.vector.tensor_scalar_mul(rden, rden, lamv)
                    bb = work.tile([128, QB], F32, tag=f"b{a}", name=f"bb{a}",
                                   bufs=1)
                    nc.gpsimd.partition_broadcast(bb, rden)
                    if a == 0:
                        # e0 scaling runs early (overlaps scores of attn 1)
                        for st in range(NST):
                            nc.vector.tensor_mul(eT[0][:, st, :],
                                                 eT[0][:, st, :], bb)
                    else:
                        # per-st scale+subtract so out-matmuls start after
                        # the first s-tiles instead of after the whole train
                        for st in range(NST):
                            nc.vector.tensor_mul(eT[1][:, st, :],
                                                 eT[1][:, st, :], bb)
                            nc.vector.tensor_sub(eT[0][:, st, :],
                                                 eT[0][:, st, :],
                                                 eT[1][:, st, :])
                # out[q,d] = sum_s diffT[s,q]^T-stat @ v[s,d].
                # d-halves: all 4 q-tiles share one v pass per half
                # (PSUM: 4x one-bank accumulators per half)
                nqt = QB // 128
                outs_t = [work.tile([128, D], F32, tag=f"outs{j}",
                                    name=f"outs{j}", bufs=1)
                          for j in range(nqt)]
                for dh in range(2):
                    po = [psout.tile([128, 512], F32, tag="out",
                                     name=f"po{dh}_{j}") for j in range(nqt)]
                    for st in range(NST):
                        vt = vstream.tile([128, 512], F32R, tag="v",
                                          name="vt")
                        nc.sync.dma_start(
                            out=vt,
                            in_=v_st[st][:, dh * 512:(dh + 1) * 512].bitcast(
                                F32R))
                        for j in range(nqt):
                            nc.tensor.matmul(
                                po[j],
                                lhsT=eT[0][:, st, j * 128:(j + 1) * 128],
                                rhs=vt,
                                start=(st == 0), stop=(st == NST - 1))
                    for j in range(nqt):
                        nc.vector.tensor_copy(
                            outs_t[j][:, dh * 512:(dh + 1) * 512], po[j])
                # RMSNorm + final scale on SBUF
                for j in range(nqt):
                    ssq = work.tile([128, 1], F32, tag="ssq", name="ssq")
                    sqv = work.tile([128, D], F32, tag="sq", name="sqv",
                                    bufs=1)
                    nc.scalar.activation(sqv, outs_t[j], AF.Square,
                                         accum_out=ssq)
                    rms = work.tile([128, 1], F32, tag="rms", name="rms")
                    nc.scalar.activation(rms, ssq, AF.Sqrt,
                                         scale=1.0 / D, bias=eps_sb)
                    rr = work.tile([128, 1], F32, tag="rr", name="rr")
                    nc.vector.reciprocal(rr, rms)
                    nc.vector.tensor_scalar_mul(rr, rr, 1.0 - LAMBDA_INIT)
                    nc.vector.tensor_scalar_mul(outs_t[j], outs_t[j], rr)
                    nc.sync.dma_start(out=out_v[bi * nqt + j], in_=outs_t[j])

        q12_cm.__exit__(None, None, None)
        vstream_cm.__exit__(None, None, None)
        kstream_cm.__exit__(None, None, None)
        singles_cm.__exit__(None, None, None)

    nc.finalize()
    return nc


def get_nc():
    if "nc" not in _CACHE:
        _CACHE["nc"] = _build_nc()
    return _CACHE["nc"]


def make_in_maps(x, w_q12, w_k12, w_v, lambda_q1, lambda_k1, lambda_q2,
                 lambda_k2):
    lam_all = np.concatenate(
        [np.asarray(lambda_q1), np.asarray(lambda_q2),
         np.asarray(lambda_k1), np.asarray(lambda_k2)]
    ).astype(np.float32).reshape(1, 4 * D)
    wq_ = np.ascontiguousarray(np.asarray(w_q12, dtype=np.float32))
    wk_ = np.ascontiguousarray(np.asarray(w_k12, dtype=np.float32))
    wv_ = np.ascontiguousarray(np.asarray(w_v, dtype=np.float32))
    in_maps = []
    for c in range(NCORES):
        b, h = divmod(c, 2)
        xb = np.asarray(x[b], dtype=np.float32)
        xT_ = np.ascontiguousarray(xb.T)
        xTq_ = np.ascontiguousarray(xb[h * QH:(h + 1) * QH, :].T)
        in_maps.append({"xT": xT_, "xTq": xTq_, "wq": wq_, "wk": wk_,
                        "wv": wv_, "lams": lam_all})
    return in_maps


def kernel(x, w_q12, w_k12, w_v, lambda_q1, lambda_k1, lambda_q2, lambda_k2,
           **run_kwargs):
    nc = get_nc()
    in_maps = make_in_maps(x, w_q12, w_k12, w_v, lambda_q1, lambda_k1,
                           lambda_q2, lambda_k2)
    res = run_bass_kernel_spmd(nc, in_maps, list(range(NCORES)), **run_kwargs)
    _CACHE["last_result"] = res
    out = np.empty((B, S, D), dtype=np.float32)
    for c in range(NCORES):
        b, h = divmod(c, 2)
        out[b, h * QH:(h + 1) * QH, :] = res.results[c]["out"]
    return out



# revision 7
# speedup vs baseline: 1.3324x; 1.3324x over previous
"""DiffAttn TRN2 kernel: 8-core SPMD (batch x query-half sharding).

Algebraic restructure vs the direct formulation: fold the q/k projections
into M_a = Wq_a @ Wk_a^T (computed once on host, [D, 2D] packed), so

  scores_a = (xq @ M_a) @ x^T        (A-matmul + scores matmul)

which removes the k-projection (the largest matmul) and all K staging
through DRAM. The second softmax's lambda weighting uses the per-query
ratio c[q] = lam * den0[q] / den1[q]:

  diff * den0 = e0 - c * e1

and the leading 1/den0 normalization is skipped entirely because the
final RMSNorm is scale-invariant per row (den0 > 0 so no sign flip).
lam itself (exp(lq1.lk1) - exp(lq2.lk2) + 0.8) is folded on host.

Per core (batch b = core//2, query half h = core%2), x columns permuted
so the core's own q-half comes first (s-order permutation is harmless:
scores/v/out all iterate s-tiles consistently):

  phase 1: A12T[d,q] = sum_din M12[din,d] xqT[din,q]   (f32r)
  phase 2: v[s,e]    = sum_d  xT[d,s] wv[d,e]          (f32r -> bf16)
  phase 3: sT_a[s,q] = sum_d  xT[d,s] A_aT[d,q]; e_a = exp(scale*sT_a)
           den_a[q] via ones-matmul; e0 -= c*e1 (bf16 DVE)
           out[q,d] = sum_s e0T[s,q] v[s,d] (bf16); RMSNorm * 0.2

Everything stays SBUF-resident (xT 64K/part, A12T 64K, v-bf16 32K,
e-bf16 32K per q-block); only ~16MB of HBM reads per core.
"""

import sys

for _p in ("/opt/trn_rl_repo", "/root/.axon_site/_ro/trn_rl_repo"):
    if _p not in sys.path:
        sys.path.append(_p)

import numpy as np

import concourse.bass as bass
import concourse.mybir as mybir
from concourse import bacc
from concourse.bass_utils import run_bass_kernel_spmd
from concourse.tile import TileContext

F32 = mybir.dt.float32
F32R = mybir.dt.float32r
BF16 = mybir.dt.bfloat16
AF = mybir.ActivationFunctionType

D = 1024          # embed dim
S = 2048          # sequence length
B = 4             # batch
NCORES = 8
QH = 1024         # query rows per core (half a sequence)
QB = 512          # query block (matmul moving dim)
NQB = QH // QB    # 2
NQT = QB // 128   # 4 q-tiles per block
NDT = D // 128    # 8 contraction tiles
NST = S // 128    # 16 key tiles
NMC = 8           # m12 column chunks streamed (2*D / MCW)
MCW = 2 * D // NMC  # 256 columns per chunk
LAMBDA_INIT = 0.8
EPS = 1e-5
SCALE = float(D) ** -0.25

_CACHE = {}


def _build_nc():
    nc = bacc.Bacc("TRN2", target_bir_lowering=False, debug=False,
                   num_devices=NCORES)

    xT = nc.declare_dram_parameter("xT", [D, S], F32, isOutput=False)
    m12 = nc.declare_dram_parameter("m12", [D, 2 * D], F32, isOutput=False)
    wv = nc.declare_dram_parameter("wv", [D, D], F32, isOutput=False)
    lamc = nc.declare_dram_parameter("lamc", [1, 1], F32, isOutput=False)
    out = nc.declare_dram_parameter("out", [QH, D], F32, isOutput=True)

    xT_v = xT.ap().rearrange("(dt p) s -> p dt s", p=128).bitcast(F32R)
    m12_v = m12.ap().rearrange("(dt p) e -> p dt e", p=128).bitcast(F32R)
    wv_v = wv.ap().rearrange("(dt p) e -> p dt e", p=128).bitcast(F32R)
    out_v = out.ap().rearrange("(t p) e -> t p e", p=128)   # [8,128,D]

    with TileContext(nc) as tc:
        singles_cm = tc.tile_pool(name="singles", bufs=1)
        singles = singles_cm.__enter__()

        lam_sb = singles.tile([1, 1], F32)
        nc.sync.dma_start(out=lam_sb, in_=lamc.ap())
        ones_f = singles.tile([128, 1], F32)
        nc.vector.memset(ones_f, 1.0)
        ones_bf = singles.tile([128, 1], BF16)
        nc.vector.tensor_copy(ones_bf, ones_f)
        eps_sb = singles.tile([128, 1], F32)
        nc.vector.memset(eps_sb, EPS)

        # ---- resident tensors (left stack) -------------------------------
        px_cm = tc.tile_pool(name="px", bufs=1)
        px = px_cm.__enter__()
        pa12_cm = tc.tile_pool(name="pa12", bufs=1)
        pa12 = pa12_cm.__enter__()
        pvsb_cm = tc.tile_pool(name="pvsb", bufs=1)
        pvsb = pvsb_cm.__enter__()

        xT_sb = px.tile([128, NDT, S], F32R)
        a12_sb = pa12.tile([128, 2 * NDT, QH], F32R)
        v_sb = pvsb.tile([128, NST, D], BF16)

        # m12 streamed in NMC column chunks (right stack, freed after ph.1)
        pm_cm = tc.tile_pool(name="pm", bufs=4, side="right")
        pm = pm_cm.__enter__()
        psa_cm = tc.tile_pool(name="psa", bufs=2, space="PSUM")
        psa = psa_cm.__enter__()

        # DMA issue order = HBM arrival order. Critical path: m12 chunk 0
        # + all q-columns of xT feed the first A-matmul chains; wv and the
        # non-q xT columns are only needed from phase 2 onward.
        mts = {}
        mts[0] = pm.tile([128, NDT, MCW], F32R, tag="m12", name="mt", bufs=4)
        nc.sync.dma_start(out=mts[0], in_=m12_v[:, :, 0:MCW])
        for dt in range(NDT):
            nc.sync.dma_start(out=xT_sb[:, dt, 0:QH], in_=xT_v[:, dt, 0:QH])

        # ---- phase 1: A12T[d, q] -----------------------------------------
        for mc in range(NMC):
            if mc in mts:
                mt = mts[mc]
            else:
                mt = pm.tile([128, NDT, MCW], F32R, tag="m12", name="mt",
                             bufs=4)
                nc.sync.dma_start(out=mt,
                                  in_=m12_v[:, :, mc * MCW:(mc + 1) * MCW])
            for ti in range(MCW // 128):
                t = mc * (MCW // 128) + ti
                pa = psa.tile([128, QH], F32, name="pa")
                for dt in range(NDT):
                    lhsT = mt[:, dt, ti * 128:(ti + 1) * 128]
                    for qc in range(QH // 512):
                        nc.tensor.matmul(
                            pa[:, qc * 512:(qc + 1) * 512],
                            lhsT=lhsT,
                            rhs=xT_sb[:, dt, qc * 512:(qc + 1) * 512],
                            start=(dt == 0), stop=(dt == NDT - 1))
                nc.scalar.copy(a12_sb[:, t, :], pa)
            if mc == 0:
                # queue phase-2/3 weights behind the phase-1 critical head
                for dt in range(NDT):
                    nc.sync.dma_start(out=xT_sb[:, dt, QH:S],
                                      in_=xT_v[:, dt, QH:S])

        psa_cm.__exit__(None, None, None)
        pm_cm.__exit__(None, None, None)

        pwv_cm = tc.tile_pool(name="pwv", bufs=1, side="right")
        pwv = pwv_cm.__enter__()
        psv_cm = tc.tile_pool(name="psv", bufs=2, space="PSUM")
        psv = psv_cm.__enter__()

        wv_sb = pwv.tile([128, NDT, D], F32R)
        for dt in range(NDT):
            nc.sync.dma_start(out=wv_sb[:, dt, :], in_=wv_v[:, dt, :])

        # ---- phase 2: v[s, e] -> bf16, SBUF-resident ---------------------
        for st in range(NST):
            pv = psv.tile([128, D], F32, name="pv")
            for dt in range(NDT):
                lhsT = xT_sb[:, dt, st * 128:(st + 1) * 128]
                for oc in range(D // 512):
                    nc.tensor.matmul(
                        pv[:, oc * 512:(oc + 1) * 512],
                        lhsT=lhsT,
                        rhs=wv_sb[:, dt, oc * 512:(oc + 1) * 512],
                        start=(dt == 0), stop=(dt == NDT - 1))
            nc.scalar.copy(v_sb[:, st, :], pv)

        psv_cm.__exit__(None, None, None)
        pwv_cm.__exit__(None, None, None)

        # ---- phase 3: attention ------------------------------------------
        with tc.tile_pool(name="eblk", bufs=1) as eblk, \
             tc.tile_pool(name="work", bufs=2) as work, \
             tc.tile_pool(name="pssc", bufs=2, space="PSUM") as pssc, \
             tc.tile_pool(name="psden", bufs=1, space="PSUM") as psden, \
             tc.tile_pool(name="psout", bufs=2, space="PSUM") as psout:
            for bi in range(NQB):
                qs = bi * QB
                eT = {}
                pden = {}
                for a in (0, 1):
                    eT[a] = eblk.tile([128, NST, QB], BF16,
                                      tag=f"e{a}", name=f"eT{a}")
                    for st in range(NST):
                        psc = pssc.tile([128, QB], F32, tag="sc", name="psc")
                        for dt in range(NDT):
                            nc.tensor.matmul(
                                psc,
                                lhsT=xT_sb[:, dt, st * 128:(st + 1) * 128],
                                rhs=a12_sb[:, a * NDT + dt, qs:qs + QB],
                                start=(dt == 0), stop=(dt == NDT - 1))
                        nc.scalar.activation(eT[a][:, st, :], psc, AF.Exp,
                                             scale=SCALE)
                    # denominator over s (partition axis) via ones-matmul
                    pden[a] = psden.tile([1, QB], F32, tag=f"den{a}",
                                         name=f"pden{a}")
                    for st in range(NST):
                        nc.tensor.matmul(pden[a], lhsT=ones_bf,
                                         rhs=eT[a][:, st, :],
                                         start=(st == 0), stop=(st == NST - 1))
                # c[q] = lam * den0[q] / den1[q]; e0 <- e0 - c*e1.
                # 1/den0 is never applied: RMSNorm cancels per-row scales.
                rden = work.tile([1, QB], F32, tag="rden", name="rden",
                                 bufs=1)
                nc.vector.reciprocal_approx_fast(rden, pden[1])
                nc.vector.tensor_mul(rden, rden, pden[0])
                nc.vector.tensor_scalar_mul(rden, rden, lam_sb)
                c_bf = work.tile([1, QB], BF16, tag="cbf", name="cbf", bufs=1)
                nc.vector.tensor_copy(c_bf, rden)
                bb = work.tile([128, QB], BF16, tag="bb", name="bb", bufs=1)
                nc.gpsimd.partition_broadcast(bb, c_bf)
                for st in range(NST):
                    nc.vector.tensor_mul(eT[1][:, st, :], eT[1][:, st, :], bb)
                    nc.vector.tensor_sub(eT[0][:, st, :], eT[0][:, st, :],
                                         eT[1][:, st, :])
                # out[q, d] = sum_s e0T[s, q] v[s, d], then RMSNorm
                for j in range(NQT):
                    po = psout.tile([128, D], F32, tag="out", name=f"po{j}")
                    for st in range(NST):
                        lhsT = eT[0][:, st, j * 128:(j + 1) * 128]
                        for dh in range(2):
                            nc.tensor.matmul(
                                po[:, dh * 512:(dh + 1) * 512],
                                lhsT=lhsT,
                                rhs=v_sb[:, st, dh * 512:(dh + 1) * 512],
                                start=(st == 0), stop=(st == NST - 1))
                    outs = work.tile([128, D], F32, tag="outs", name="outs",
                                     bufs=1)
                    nc.vector.tensor_copy(outs, po)
                    ssq = work.tile([128, 1], F32, tag="ssq", name="ssq")
                    sqv = work.tile([128, D], F32, tag="sq", name="sqv",
                                    bufs=1)
                    nc.scalar.activation(sqv, outs, AF.Square, accum_out=ssq)
                    rms = work.tile([128, 1], F32, tag="rms", name="rms")
                    nc.scalar.activation(rms, ssq, AF.Sqrt,
                                         scale=1.0 / D, bias=eps_sb)
                    rr = work.tile([128, 1], F32, tag="rr", name="rr")
                    nc.vector.reciprocal(rr, rms)
                    nc.vector.tensor_scalar_mul(rr, rr, 1.0 - LAMBDA_INIT)
                    nc.vector.tensor_scalar_mul(outs, outs, rr)
                    nc.sync.dma_start(out=out_v[bi * NQT + j], in_=outs)

        pvsb_cm.__exit__(None, None, None)
        pa12_cm.__exit__(None, None, None)
        px_cm.__exit__(None, None, None)
        singles_cm.__exit__(None, None, None)

    nc.finalize()
    return nc


def get_nc():
    if "nc" not in _CACHE:
        _CACHE["nc"] = _build_nc()
    return _CACHE["nc"]


def make_in_maps(x, w_q12, w_k12, w_v, lambda_q1, lambda_k1, lambda_q2,
                 lambda_k2):
    wq = np.asarray(w_q12, dtype=np.float64)
    wk = np.asarray(w_k12, dtype=np.float64)
    m1 = wq[:, :D] @ wk[:, :D].T
    m2 = wq[:, D:] @ wk[:, D:].T
    m12_ = np.ascontiguousarray(
        np.concatenate([m1, m2], axis=1).astype(np.float32))
    wv_ = np.ascontiguousarray(np.asarray(w_v, dtype=np.float32))
    lam1 = np.exp(np.float64(lambda_q1) @ np.float64(lambda_k1))
    lam2 = np.exp(np.float64(lambda_q2) @ np.float64(lambda_k2))
    lam_ = np.array([[lam1 - lam2 + LAMBDA_INIT]], dtype=np.float32)
    in_maps = []
    for c in range(NCORES):
        b, h = divmod(c, 2)
        xb = np.asarray(x[b], dtype=np.float32)
        # own q-half rows first so the kernel's q columns are 0:QH
        xp = np.concatenate([xb[h * QH:(h + 1) * QH, :],
                             xb[(1 - h) * QH:(2 - h) * QH, :]], axis=0)
        xT_ = np.ascontiguousarray(xp.T)
        in_maps.append({"xT": xT_, "m12": m12_, "wv": wv_, "lamc": lam_})
    return in_maps


def kernel(x, w_q12, w_k12, w_v, lambda_q1, lambda_k1, lambda_q2, lambda_k2,
           **run_kwargs):
    nc = get_nc()
    in_maps = make_in_maps(x, w_q12, w_k12, w_v, lambda_q1, lambda_k1,
                           lambda_q2, lambda_k2)
    res = run_bass_kernel_spmd(nc, in_maps, list(range(NCORES)), **run_kwargs)
    _CACHE["last_result"] = res
    out = np.empty((B, S, D), dtype=np.float32)
    for c in range(NCORES):
        b, h = divmod(c, 2)
        out[b, h * QH:(h + 1) * QH, :] = res.results[c]["out"]
    return out
